# revision 59
# baseline (speedup 1.0000x reference)
"""GCN diag-encoder (2-layer SpMM) on 8 Trainium2 NeuronCores.

Strategy: the sparse adjacency (640K edges over 10K nodes, ~0.64% dense) is
materialized as a dense A^T matrix on the host; each per-layer
  out[dst] = sum_e vals[e] * x[src[e]]        (segment-sum SpMM)
becomes dense TensorEngine matmuls.  Each core owns a 1250-wide dst slice of
A^T (padded to 1280, uint8-quantized with one shared scale per group of 10
similarly-sized columns -- columns are sorted by max so grouping costs no
precision, and the dequant scale becomes a single per-partition vector).

The A^T shard streams from HBM exactly ONCE, as RAW u8 (the DMA cost model
bills by destination-side bytes, so a u8->u8 copy costs half of the inline
u8->f16 cast DMA), into a 13.1MB SBUF-resident buffer.  Each k-tile group is
then widened u8->f16 into staging tiles by the otherwise-idle compute
engines (DVE tensor_copy in 2x mode ~0.52ns/elem, ACT copy ~0.83, Pool
tensor_copy ~1.39): the 3-way split sustains one group per ~1.6us, faster
than the PE consumes them (2.13us/group), so the Tensor engine never stalls
and stays at its full 2.4GHz p-state (a stall drops it to 1.2GHz for 3us).
Layer 2 re-casts from the resident u8 -- zero layer-2 A traffic.

All DMAs ride the sync(SP) ring: the SP sequencer has no compute work, so
every descriptor is issued far ahead of time and fires on data-readiness,
keeping DMA issue latency off both the compute sequencers and the critical
eviction -> AllGather -> x1 chain.  PSUM accumulators are split into
per-bank tiles so layer-2's 512-col chunks carry no false cross-chunk deps.
Dummy zero-matmuls into a scratch PSUM bank bridge the PE p-state across
the pipeline-fill window; x0 rides in per-group chunks early and paired
chunks later so the a-group delivery cadence outruns the PE.  The last few
groups accumulate bank-major with seg-major casts, closing psum banks early
so the eviction chain overlaps real work, and the list scheduler then slots
layer-2's own-rank k-tiles (fed straight from the eviction buffer) into
layer-1's residual delivery waits.

Layer 1 runs A-stationary (psum [dst slot, feat]): eviction is 3 wide
fused tanh+dequant ACT ops straight into the AllGather bounce.  Layer 2
(X-stationary; psum [feat, dst]) dequant + transpose happen on the host.
Src nodes use a padded rank-block ordering (rank r owns slots
r*1280..r*1280+1279, permuted by the per-core column sort) so layer 2's
AllGathered activations line up with the SAME A arrangement layer 1 uses.
W0 is folded into x on the host; W1 is skipped on device when all-ones.
"""

import numpy as np
import ml_dtypes

N = 10000          # nodes
D = 128            # feature dim
NCORES = 8
S = 1250           # dst nodes per core
SP = 1280          # padded dst per core (10 tiles of 128)
NT = 10            # dst tiles per core
KT = 80            # contraction k-tiles (padded src rows = 10240)
NPAD = KT * 128    # 10240
GSIZES = (4,) * 19 + (2, 2)   # k-tiles per group (small tail for early stops)
CHUNKS = ((0, 512), (512, 1024), (1024, 1280))   # psum bank chunks (L1)
CHUNKS2 = ((0, 512), (512, 994), (994, 1250))  # L2: pads trimmed; all
# chunks >=256 cols so no out-DMA descriptor falls under the 512B 2x penalty
BF16 = ml_dtypes.bfloat16

_PROG_CACHE = {}


def _groups(gsizes=GSIZES):
    out = []
    k0 = 0
    for sz in gsizes:
        out.append((k0, k0 + sz))
        k0 += sz
    assert k0 == KT
    return out


def _build_program(nocc=False, sbufs=6, ndum0=58, ndum=0, l1tail=3,
                   ndumt=0, x0plan=(6, 8, 10, 12, 14, 16, 18), ntail=5,
                   w1_ones=True, gsizes=GSIZES, nocc_full8=False):
    import concourse.bacc as bacc
    import concourse.mybir as mybir
    from concourse import tile

    f32 = mybir.dt.float32
    f16 = mybir.dt.float16
    u8 = mybir.dt.uint8
    grps = _groups(gsizes)
    maxg = max(k1 - k0 for k0, k1 in grps)

    nc = bacc.Bacc(
        "TRN2",
        target_bir_lowering=False,
        debug=False,
        enable_asserts=False,
        num_devices=1 if nocc else NCORES,
    )

    a = nc.dram_tensor("a", [KT, 128, S], u8, kind="ExternalInput").ap()
    x0 = nc.dram_tensor("x0", [128, NPAD], f16, kind="ExternalInput").ap()
    # per-partition dequant scale (shared by the 10 dst slots per partition)
    csc = nc.dram_tensor("csc", [128, 1], f32, kind="ExternalInput").ap()
    # broadcast W1 row tiled 4x (only read when not w1_ones)
    w1b = nc.dram_tensor("w1b", [128, 512], f16, kind="ExternalInput").ap()
    out = nc.dram_tensor("out", [128, SP], f16, kind="ExternalOutput").ap()

    with tile.TileContext(nc) as tc:
        with (
            tc.tile_pool(name="xp", bufs=1) as xp,
            tc.tile_pool(name="stg", bufs=sbufs) as stg,
            tc.tile_pool(name="ev", bufs=1) as ev,
            tc.tile_pool(name="ps", bufs=1, space="PSUM") as ps,
            tc.tile_pool(name="dr", bufs=1, space="DRAM") as dr,
        ):
            # the full A^T shard, raw u8, resident for both layers; only
            # the 1250 real dst columns are DMAed.  The 30 pad columns per
            # k-tile hold SBUF junk: u8->f16 casts keep it finite, layer 1
            # accumulates it into psum partitions whose evictions only feed
            # x1 pad slots, and those multiply the host-zeroed pad src rows
            # in layer 2 -- so the junk never reaches the output.
            au = xp.tile([128, KT * SP], u8, tag="au")
            x0s = xp.tile([128, NPAD], f16, tag="x0s")
            x1s = xp.tile([128, NPAD], f16, tag="x1s")
            cscs = xp.tile([128, 1], f32, tag="cscs")
            w1s = xp.tile([128, 512], f16, tag="w1s")
            zl = xp.tile([128, 512], f16, tag="zl")
            warm = xp.tile([128, 1], f32, tag="warm")
            nc.sync.dma_start(cscs[:], csc)
            if not w1_ones:
                nc.sync.dma_start(w1s[:], w1b)
            nc.vector.memset(zl[:], 0.0)
            # pre-load the ACT tanh table so the layer-1 eviction doesn't
            # pay the table load on the critical path
            nc.scalar.activation(
                warm[:], zl[:, 0:1], mybir.ActivationFunctionType.Tanh
            )

            agin = dr.tile([128, SP], f16)
            if nocc:
                agout = dr.tile([NCORES * 128, SP], f16)
            else:
                agout = dr.tile([NCORES * 128, SP], f16,
                                addr_space="Shared")

            def fetch_group(gi):
                """DMA group gi of A (raw u8, real columns only)."""
                k0, k1 = grps[gi]
                nk = k1 - k0
                dst = au[:, k0 * SP:k1 * SP].rearrange(
                    "p (k j) -> p k j", k=nk
                )[:, :, 0:S]
                nc.sync.dma_start(dst, a[k0:k1].rearrange("k p j -> p k j"))

            def cast_group(gi, use_act=True):
                """Widen group gi u8->f16 into a staging tile, split across
                the idle compute engines in 128-col spans."""
                k0, k1 = grps[gi]
                nw = (k1 - k0) * NT  # 128-col windows
                sb = stg.tile([128, maxg * SP], f16, tag="stg")
                if use_act:
                    dw = int(round(0.55 * nw))   # DVE share (0.52 ns/elem)
                    aw = int(round(0.275 * nw))  # ACT share (0.83 ns/elem)
                else:
                    dw = int(round(0.7 * nw))
                    aw = 0
                pw0 = dw + aw                 # Pool takes the rest (1.39)
                for eng, w0, w1 in (("d", 0, dw), ("a", dw, pw0),
                                    ("p", pw0, nw)):
                    if w1 <= w0:
                        continue
                    c0, c1 = w0 * 128, w1 * 128
                    src = au[:, k0 * SP + c0:k0 * SP + c1]
                    if eng == "d":
                        nc.vector.tensor_copy(sb[:, c0:c1], src)
                    elif eng == "a":
                        nc.scalar.copy(sb[:, c0:c1], src)
                    else:
                        nc.gpsimd.tensor_copy(sb[:, c0:c1], src)
                return sb

            # per-bank PSUM accumulators (no false cross-chunk deps)
            ps1 = [ps.tile([128, b1 - b0], f32, tag=f"acc1_{b0}",
                           name=f"acc1_{b0}") for b0, b1 in CHUNKS]
            ps2 = [ps.tile([128, b1 - b0], f32, tag=f"acc2_{b0}",
                           name=f"acc2_{b0}") for b0, b1 in CHUNKS2]
            warmps = ps.tile([128, 128], f32, tag="warmps")

            def p1(t):
                """psum1 window for dst tile t."""
                ci = 0 if t < 4 else (1 if t < 8 else 2)
                o = (t - (0, 4, 8)[ci]) * 128
                return ps1[ci][:, o:o + 128]

            # ---- layer 1 (A-stationary; psum1 is [dst slot, feat]) ----
            for ci, (b0, b1) in enumerate(CHUNKS):
                nc.tensor.matmul(
                    ps1[ci][:], zl[:, 0:128], zl[:, 0:b1 - b0],
                    start=True, stop=False,
                )
            # keep PE busy through the pipeline fill so the first real
            # matmuls run at full p-state
            for _ in range(ndum0):
                nc.tensor.matmul(warmps[:], zl[:, 0:128], zl[:, 0:128],
                                 start=True, stop=True)
            # DMA issue order = pool service order (among ready): interleave
            # x0 with the first a-groups, then batch the rest of x0 into two
            # chunks so the a-group cadence drops to 1.82us/group and the
            # stream runs ahead of the PE.
            def x0_chunk(g0, g1):
                c0, c1 = grps[g0][0] * 128, grps[g1 - 1][1] * 128
                nc.sync.dma_start(x0s[:, c0:c1], x0[:, c0:c1])

            nsingle = x0plan[0]
            bounds = [nsingle] + list(x0plan[1:]) + [len(grps)]
            for gi in range(len(grps)):
                # a-group first: it gates the cast pipeline; the x0 chunk
                # only feeds matmuls that run later
                fetch_group(gi)
                if gi < nsingle:
                    x0_chunk(gi, gi + 1)
                elif gi in bounds[:-1]:
                    bi = bounds.index(gi)
                    x0_chunk(gi, bounds[bi + 1])

            nt2 = ntail  # trailing groups processed bank-major (layer 2)
            l1cut = l1tail if l1tail > 0 else 1
            for gi, (k0, k1) in enumerate(grps[:-l1cut]):
                sb = cast_group(gi)
                for k in range(k0, k1):
                    kk = k - k0
                    rhs = x0s[:, k * 128:(k + 1) * 128]
                    for t in range(NT):
                        nc.tensor.matmul(
                            p1(t),
                            sb[:, kk * SP + t * 128:kk * SP + (t + 1) * 128],
                            rhs,
                            start=False, stop=False,
                        )
            # tail: run bank-major over the last l1cut groups so bank 0
            # closes before layer 1 ends and the eviction -> AllGather
            # -> x1 chain overlaps the remaining real matmuls
            def cast_group_segwise(gi):
                """Tail cast: one op per (k-tile, t-segment) in seg-major
                order so the t0..3 windows are ready right after the DMA."""
                k0, k1 = grps[gi]
                sb = stg.tile([128, maxg * SP], f16, tag="stg")
                engs = [nc.vector.tensor_copy, nc.scalar.copy,
                        nc.gpsimd.tensor_copy]
                ei = 0
                for s0, s1 in ((0, 512), (512, 1024), (1024, SP)):
                    for k in range(k0, k1):
                        kk = k - k0
                        c0, c1 = kk * SP + s0, kk * SP + s1
                        engs[ei % 3](sb[:, c0:c1], au[:, k0 * SP + c0:
                                                      k0 * SP + c1])
                        ei += 1
                return sb

            tail1 = list(range(len(grps) - l1cut, len(grps)))
            tsb = {gi: cast_group_segwise(gi) for gi in tail1}
            for ci, (t0, t1) in enumerate(((0, 4), (4, 8), (8, NT))):
                for gi in tail1:
                    k0, k1 = grps[gi]
                    if ci == 0 and gi != tail1[0]:
                        # absorb the end-of-stream delivery stall with
                        # keep-warm matmuls so the PE p-state holds
                        for _ in range(ndumt):
                            nc.tensor.matmul(warmps[:], zl[:, 0:128],
                                             zl[:, 0:128],
                                             start=True, stop=True)
                    for k in range(k0, k1):
                        kk = k - k0
                        for t in range(t0, t1):
                            nc.tensor.matmul(
                                p1(t),
                                tsb[gi][:, kk * SP + t * 128:
                                        kk * SP + (t + 1) * 128],
                                x0s[:, k * 128:(k + 1) * 128],
                                start=False,
                                stop=(k == KT - 1 and t == t1 - 1),
                            )

            # evict layer 1: x1 = tanh(cs * psum1) [* W1] on ACT, one wide
            # op per bank (the shared-per-partition scale allows it), each
            # bank's agin chunk in flight while later banks still compute.
            agin_sb = ev.tile([128, SP], f16, tag="agin")
            for ci, (b0, b1) in enumerate(CHUNKS):
                nc.scalar.activation(
                    agin_sb[:, b0:b1], ps1[ci][:],
                    mybir.ActivationFunctionType.Tanh,
                    scale=cscs[:, 0:1],
                )
                if not w1_ones:
                    nc.vector.tensor_mul(
                        agin_sb[:, b0:b1], agin_sb[:, b0:b1],
                        w1s[:, 0:b1 - b0]
                    )
                nc.sync.dma_start(agin[:, b0:b1], agin_sb[:, b0:b1])

            if nocc:
                if nocc_full8:
                    # validation twin: every rank block = own shard
                    for r in range(NCORES):
                        nc.scalar.dma_start(
                            agout[r * 128:(r + 1) * 128, :], agin[:]
                        )
                else:
                    # timing twin: skip the bounce; x1 reads agin directly
                    pass
            else:
                nc.gpsimd.collective_compute(
                    "AllGather",
                    mybir.AluOpType.bypass,
                    replica_groups=[list(range(NCORES))],
                    ins=[agin.opt()],
                    outs=[agout.opt()],
                )

            # bridge the PE p-state across the eviction + AllGather window
            for _ in range(ndum):
                nc.tensor.matmul(warmps[:], zl[:, 0:128], zl[:, 0:128],
                                 start=True, stop=True)

            # agout rank blocks laid side by side in the free dim are exactly
            # layer-2's lhsT tiles in the same padded rank-block order A uses.
            # Rank 0 lands per bank chunk so layer 2's first k-tiles unblock
            # as soon as the first agin chunk is through.
            use_agin = nocc and not nocc_full8
            if not use_agin:
                # rank 0 lands per bank chunk so layer 2's first k-tiles
                # unblock as soon as the first agin chunk is through
                for b0, b1 in CHUNKS:
                    nc.sync.dma_start(x1s[:, b0:b1], agout[0:128, b0:b1])
            # (timing twin reads rank 0 straight out of agin_sb as lhsT)
            for r in range(1, NCORES):
                # twin: copy from the eviction buffer (SBUF) -- the agin
                # DRAM writes still happen (and are billed) as in the real
                # program, but the x1 path need not wait for them
                src = (agin_sb[:] if use_agin
                       else agout[r * 128:(r + 1) * 128, :])
                nc.sync.dma_start(x1s[:, r * SP:(r + 1) * SP], src)

            def l2_lhsT(k):
                if use_agin and k < NT:
                    return agin_sb[:, k * 128:(k + 1) * 128]
                return x1s[:, k * 128:(k + 1) * 128]

            # ---- layer 2 (X-stationary; psum2 is [feat, dst]) ----
            ob = ev.tile([128, SP], f16, tag="ob")
            first = True
            for gi, (k0, k1) in enumerate(grps[:-nt2]):
                # first L2 groups skip ACT so the scheduler can't queue
                # their casts ahead of the layer-1 eviction
                sb = cast_group(gi, use_act=gi >= 2)
                for k in range(k0, k1):
                    kk = k - k0
                    lhsT = l2_lhsT(k)
                    for ci, (b0, b1) in enumerate(CHUNKS2):
                        nc.tensor.matmul(
                            ps2[ci][:],
                            lhsT,
                            sb[:, kk * SP + b0: kk * SP + b1],
                            start=first, stop=False,
                        )
                    first = False
            # tail: bank-major over the last ntail groups so each psum2
            # bank completes early and its eviction overlaps the rest
            tail = list(range(len(grps) - nt2, len(grps)))
            tsb2 = {gi: cast_group(gi) for gi in tail}
            for ci, (b0, b1) in enumerate(CHUNKS2):
                for gi in tail:
                    k0, k1 = grps[gi]
                    for k in range(k0, k1):
                        kk = k - k0
                        nc.tensor.matmul(
                            ps2[ci][:],
                            l2_lhsT(k),
                            tsb2[gi][:, kk * SP + b0: kk * SP + b1],
                            start=False, stop=(k == KT - 1),
                        )
                nc.vector.tensor_copy(ob[:, b0:b1], ps2[ci][:])
                nc.sync.dma_start(out[:, b0:b1], ob[:, b0:b1])

    nc.compile()
    return nc


def get_program(nocc=False, sbufs=6, ndum0=58, ndum=0, l1tail=3,
                ndumt=0, x0plan=(6, 8, 10, 12, 14, 16, 18), ntail=5,
                w1_ones=True, gsizes=GSIZES, nocc_full8=False):
    key = ("nc", nocc, sbufs, ndum0, ndum, l1tail, ndumt, tuple(x0plan),
           ntail, w1_ones, tuple(gsizes), nocc_full8)
    if key not in _PROG_CACHE:
        _PROG_CACHE[key] = _build_program(nocc, sbufs, ndum0, ndum, l1tail,
                                          ndumt, x0plan, ntail, w1_ones,
                                          gsizes, nocc_full8)
    return _PROG_CACHE[key]


def _slot_maps():
    """Slot q = t*128+p holds the sorted column i: partitions 0..97 hold
    10 columns each (i = p*10+t), partitions 98..127 hold 9 (t=9 is pad),
    so all padding lands contiguously at slots 1250..1279."""
    q = np.arange(SP)
    p = q % 128
    t = q // 128
    i = np.where(p < 98, p * NT + t, 980 + (p - 98) * 9 + t)
    vld = (p < 98) | (t < 9)
    return q, p, t, i, vld


def _perms(AT):
    """Per-core descending-colmax orderings + node slot map."""
    orders = []
    for c in range(NCORES):
        colmax = AT[:, c * S:(c + 1) * S].max(axis=0)
        orders.append(np.argsort(-colmax, kind="stable"))
    q, p, t, i, vld = _slot_maps()
    node2 = np.zeros(NPAD, np.int64)
    valid2 = np.zeros(NPAD, bool)
    for r in range(NCORES):
        node2[r * SP + q[vld]] = r * S + orders[r][i[vld]]
        valid2[r * SP + q[vld]] = True
    return orders, node2, valid2


def build_in_maps(x, src, dst, vals, W):
    """Host-side prep: dense A^T shard (u8, shared scale per 10 sorted
    columns) + x0 with rows in the permuted slot order."""
    import scipy.sparse as sp

    x = np.asarray(x, np.float32)
    src = np.asarray(src, np.int64)
    dst = np.asarray(dst, np.int64)
    vals = np.asarray(vals, np.float32)
    W = np.asarray(W, np.float32)

    # A[dst, src] = sum of vals  ->  we build AT[src, dst]
    AT = sp.coo_matrix((vals, (src, dst)), shape=(N, N)).toarray()
    orders, node2, valid2 = _perms(AT)
    q, p, t, i, vld = _slot_maps()

    xw = x * W[0][None, :]
    x0p = np.zeros((NPAD, D), np.float32)
    x0p[valid2] = xw[node2[valid2]]
    x0h = np.ascontiguousarray(
        x0p.reshape(KT, 128, D).transpose(1, 0, 2).reshape(128, KT * D)
    ).astype(np.float16)

    w1brow = np.ascontiguousarray(
        np.tile(W[1][None, :], (128, 4))
    ).astype(np.float16)

    in_maps = []
    steps = []
    for c in range(NCORES):
        ATc = AT[:, c * S:(c + 1) * S]  # [N, S] float32
        order = orders[c]
        colmax = np.maximum(ATc.max(axis=0), 1e-9)
        # shared scale per partition: the first (largest) column of the
        # partition's group (10 cols for p<98, 9 for p>=98)
        g0 = np.where(np.arange(128) < 98,
                      np.arange(128) * NT,
                      980 + (np.arange(128) - 98) * 9)
        s = np.maximum(colmax[order[g0]], 1e-9).astype(np.float32)
        step = s / 255.0                       # [128] dequant step
        Aq = np.zeros((N, SP), np.uint8)
        Aq[:, q[vld]] = np.clip(
            np.rint(ATc[:, order[i[vld]]] * (1.0 / step[p[vld]])[None, :]),
            0, 255,
        ).astype(np.uint8)
        Ap = np.zeros((NPAD, SP), np.uint8)
        # rows of Aq are global src nodes; node2 maps slots to global nodes
        Ap[valid2] = Aq[node2[valid2]]
        steps.append(step)
        a3 = np.ascontiguousarray(Ap.reshape(KT, 128, SP)[:, :, :S])
        in_maps.append(
            {
                "a": a3,
                "x0": x0h,
                "csc": np.ascontiguousarray(step.reshape(128, 1)),
                "w1b": w1brow,
            }
        )
    return in_maps, steps, orders


def assemble_output(results, steps, orders):
    q, p, t, i, vld = _slot_maps()
    outs = []
    for c in range(NCORES):
        ot = np.asarray(results[c]["out"], np.float32)  # [128, SP] feat-major
        oc = np.zeros((S, D), np.float32)
        oc[orders[c][i[vld]], :] = (ot[:, q[vld]] * steps[c][p[vld]][None, :]).T
        outs.append(oc)
    return np.ascontiguousarray(np.concatenate(outs, axis=0))


def kernel(x, src, dst, vals, W):
    from concourse import bass_utils

    w1_ones = bool(np.all(np.asarray(W)[1] == 1.0))
    nc = get_program(w1_ones=w1_ones)
    in_maps, steps, orders = build_in_maps(x, src, dst, vals, W)
    # The axon terminal can wedge when a different program was loaded
    # earlier in its lifetime; after the crash the terminal restarts and a
    # retry succeeds.  Back off progressively to ride out the restart.
    import time as _time

    last_err = None
    for sleep_s in (10.0, 30.0, 60.0, 0.0):
        try:
            res = bass_utils.run_bass_kernel_spmd(
                nc, in_maps, core_ids=list(range(NCORES))
            )
            return assemble_output(res.results, steps, orders)
        except Exception as e:  # noqa: BLE001
            last_err = e
            _time.sleep(sleep_s)
    raise last_err
